# revision 37
# baseline (speedup 1.0000x reference)
"""MoE (B=2048, D=1024, H=4096, E=8, top-2) — Trainium2 Bass kernel, 8 NeuronCores.

Strategy (expert-parallel, sparse token routing):
  * Host: gating (x @ Wg + bg, top-2, softmax) — 0.01% of the FLOPs. The
    token->expert routing IS the sharding step: core e receives the tokens
    assigned to expert e (gathered + padded to a common capacity C).
  * Device (per core e): outT = (gelu(x_e @ w1[e] + b1[e]) @ w2[e] + b2[e]).T
    computed in bf16 with fp32 PSUM accumulation, fully transposed layout so
    no on-device transposes are needed.
  * Host: weighted scatter-combine of the two expert outputs per token.

The dense reference computes all 8 experts per token; only the top-2 survive
the gates-weighted combine, so routing does 4x less matmul work.
"""

import json
import math

import numpy as np
from ml_dtypes import bfloat16

B, D, H, E, TOP_K = 2048, 1024, 4096, 8, 2
NCORES = 8
DBLK, HBLK = D // 128, H // 128  # 8, 32

# w1 column chunks (small first so the PE can start early; sums to H)
_W1SIZES = [128, 128, 256, 512, 1024, 1024, 1024]
W1CHUNKS = []
_c = 0
for _s in _W1SIZES:
    W1CHUNKS.append((_c, _s))
    _c += _s
W1CIDX = {}  # h-block -> (chunk j, first h-block of chunk, chunk col size)
for _j, (_cs, _sz) in enumerate(W1CHUNKS):
    for _h in range(_cs // 128, (_cs + _sz) // 128):
        W1CIDX[_h] = (_j, _cs // 128, _sz)

TRACE = False  # test.py sets this to capture an NTFF profile / exec_time_ns
LAST_RESULTS = {}  # test.py reads exec_time_ns etc. from here


def _gate(x, Wg, bg):
    """Mirror of the reference gating math in numpy float32."""
    logits = (x @ Wg + bg).astype(np.float32)  # [B, E]
    rows = np.arange(B)
    i1 = np.argmax(logits, axis=1)
    v1 = logits[rows, i1]
    masked = logits.copy()
    masked[rows, i1] = -np.inf
    i2 = np.argmax(masked, axis=1)
    v2 = masked[rows, i2]
    # softmax over the top-2 values (v1 >= v2)
    e2 = np.exp((v2 - v1).astype(np.float32))
    denom = (np.float32(1.0) + e2).astype(np.float32)
    g1 = (np.float32(1.0) / denom).astype(np.float32)
    g2 = (e2 / denom).astype(np.float32)
    gates = np.zeros((B, E), np.float32)
    gates[rows, i1] = g1
    gates[rows, i2] = g2
    top_i = np.stack([i1, i2], axis=1).astype(np.int32)

    # load-balance aux loss
    def _cv(v):
        return np.std(v, ddof=1).astype(np.float32) / (
            np.mean(v, dtype=np.float32) + np.float32(1e-6)
        )

    importance = gates.sum(axis=0, dtype=np.float32)
    m = logits.max(axis=1, keepdims=True)
    ex = np.exp(logits - m)
    sm = ex / ex.sum(axis=1, keepdims=True)
    load = sm.sum(axis=0, dtype=np.float32)
    lbl = np.asarray(_cv(importance) + _cv(load), np.float32)
    util = (gates > 0).astype(np.float32).mean(axis=0, dtype=np.float32)
    return gates, top_i, lbl, util


def _split_multi_waits(mod):
    """Legalize: this walrus accepts at most one sync-wait per instruction.

    Tile's wait assigner can attach several; hoist all but the last onto
    standalone single-wait EventSemaphore instructions on the same engine,
    immediately before the original instruction (same basic block), which
    preserves the per-engine stall semantics exactly.
    """
    for fn in mod["functions"]:
        for blk in fn["blocks"]:
            new_insts = []
            for inst in blk["instructions"]:
                si = inst.get("sync_info") or {}
                waits = si.get("on_wait") or []
                if len(waits) > 1:
                    for k, w in enumerate(waits[:-1]):
                        new_insts.append(
                            {
                                "debug": inst.get("debug", 0),
                                "engine": inst["engine"],
                                "ins": [],
                                "name": f"{inst['name']}-sw{k}",
                                "opcode": "EventSemaphore",
                                "outs": [],
                                "sync_info": {"on_update": [], "on_wait": [w]},
                            }
                        )
                    si["on_wait"] = [waits[-1]]
                new_insts.append(inst)
            blk["instructions"] = new_insts
    return mod


def _patch_serializer(nc):
    orig = nc.to_json_bytes
    nc.to_json_bytes = lambda: json.dumps(_split_multi_waits(json.loads(orig()))).encode()
    return nc


def _build(C, cblocks):
    """Bass/Tile program for one core: out.T = (gelu(x@w1+b1) @ w2 + b2).T.

    Fully transposed dataflow, so no on-device transposes are needed:
      layer 1: psum[h_blk 128, c] += w1[d_blk, h_blk].T @ xT[d_blk, c]  (8 d_blks)
      layer 2: psum[d_blk 128, c] += w2[h_blk, d_blk].T @ hT[h_blk, c]  (32 h_blks)
    Token columns are processed in c-blocks of <=512 (PSUM bank limit).
    All DRAM parameters are host-packed into [128, wide] contiguous blocks.
    """
    import concourse.bass as bass
    import concourse.mybir as mybir
    import concourse.tile as tile

    bf16, f32 = mybir.dt.bfloat16, mybir.dt.float32
    AF = mybir.ActivationFunctionType

    NB = len(cblocks)

    nc = bass.Bass()
    # Host pre-blocks everything so each DMA is ONE fully contiguous block
    # landing in one [128, wide] SBUF tile; d-blocks sit side by side in the
    # free dimension and matmuls slice their 128-column windows out. This
    # collapses ~100 DMA issues (~0.6us of sequencer time each) to ~11.
    xT_d = [
        nc.declare_dram_parameter(f"xT{bi}", [128, DBLK * cs], bf16, isOutput=False)
        for bi, (_, cs) in enumerate(cblocks)
    ]
    w1_d = [
        nc.declare_dram_parameter(
            f"w1c{j}", [128, DBLK * csz], bf16, isOutput=False
        )
        for j, (_, csz) in enumerate(W1CHUNKS)
    ]
    w2_d = [
        nc.declare_dram_parameter(f"w2q{q}", [128, (HBLK // 4) * D], bf16, isOutput=False)
        for q in range(4)
    ]
    b_d = nc.declare_dram_parameter("b", [128, HBLK + DBLK], f32, isOutput=False)
    out_d = [
        nc.declare_dram_parameter(f"out{bi}", [DBLK, 128, cs], f32, isOutput=True)
        for bi, (_, cs) in enumerate(cblocks)
    ]

    with tile.TileContext(nc) as tc:
        with (
            tc.tile_pool(name="xp", bufs=1) as xp,
            tc.tile_pool(name="w1p", bufs=1) as w1p,
            tc.tile_pool(name="w2p", bufs=1) as w2p,
            tc.tile_pool(name="bp", bufs=1) as bp,
            tc.tile_pool(name="hp", bufs=1) as hp,
            tc.tile_pool(name="op", bufs=2) as op,
            tc.tile_pool(name="ps", bufs=4, space="PSUM") as ps,
        ):
            # PE warm-up first (highest scheduler priority): ~60 junk matmuls
            # on a memset tile run during the DMA head phase so the HAM clock
            # gate is at 8/8 when real work lands.
            junk = bp.tile([128, 64], bf16, tag="junk")
            nc.vector.memset(junk[:], 0.0)
            jps = ps.tile([64, 64], f32, tag="ps1")
            for _ in range(60):
                nc.tensor.matmul(jps[:], junk[:], junk[:], start=True, stop=True)

            # ALL input loads go on the single SP HWDGE ring in exact PE
            # consumption order — the ring is FIFO, so this IS the priority
            # schedule: xT block 0, biases, w1 chunks, xT rest, then w2 in
            # four quarters (each unblocks 8 h-blocks of layer 2). Outputs go
            # on the ACT ring.
            x_sb = [None] * NB
            t = xp.tile([128, DBLK * cblocks[0][1]], bf16, tag="x0", name="x0")
            nc.sync.dma_start(out=t[:], in_=xT_d[0][:])
            x_sb[0] = t
            w1_sb = []
            b_sb = None
            for j, (_, csz) in enumerate(W1CHUNKS):
                t = w1p.tile([128, DBLK * csz], bf16, tag=f"w1_{j}", name=f"w1_{j}")
                nc.sync.dma_start(out=t[:], in_=w1_d[j][:])
                w1_sb.append(t)
                if j == 0:
                    b_sb = bp.tile([128, HBLK + DBLK], f32, tag="b")
                    nc.sync.dma_start(out=b_sb[:], in_=b_d[:])
            for bi in range(1, NB):
                t = xp.tile([128, DBLK * cblocks[bi][1]], bf16, tag=f"x{bi}", name=f"x{bi}")
                nc.sync.dma_start(out=t[:], in_=xT_d[bi][:])
                x_sb[bi] = t
            w2_sb = []
            for q in range(4):
                t = w2p.tile([128, (HBLK // 4) * D], bf16, tag=f"w2q{q}", name=f"w2q{q}")
                nc.sync.dma_start(out=t[:], in_=w2_d[q][:])
                w2_sb.append(t)

            def w1_lhsT(d, h):
                j, h0, csz = W1CIDX[h]
                base = d * csz + (h - h0) * 128
                return w1_sb[j][:, base : base + 128]

            for bi, (cstart, csize) in enumerate(cblocks):
                # layer 1: hT[h,c] = gelu(sum_d w1[d,h].T @ xT[d,c] + b1[h])
                h_tiles = []
                for h in range(HBLK):
                    acc = ps.tile([128, csize], f32, tag="ps1")
                    for d in range(DBLK):
                        nc.tensor.matmul(
                            acc[:],
                            w1_lhsT(d, h),
                            x_sb[bi][:, d * csize : (d + 1) * csize],
                            start=(d == 0),
                            stop=(d == DBLK - 1),
                        )
                    ht = hp.tile([128, csize], bf16, tag=f"h{h}")
                    nc.scalar.activation(ht[:], acc[:], AF.Gelu, bias=b_sb[:, h : h + 1])
                    h_tiles.append(ht)
                # layer 2: outT[d,c] = sum_h w2[h,d].T @ hT[h,c] + b2[d]
                for dd in range(DBLK):
                    acc2 = ps.tile([128, csize], f32, tag="ps2")
                    for h in range(HBLK):
                        hq, hr = divmod(h, HBLK // 4)
                        nc.tensor.matmul(
                            acc2[:],
                            w2_sb[hq][:, hr * D + dd * 128 : hr * D + dd * 128 + 128],
                            h_tiles[h][:],
                            start=(h == 0),
                            stop=(h == HBLK - 1),
                        )
                    ot = op.tile([128, csize], f32, tag="ot")
                    nc.scalar.activation(
                        ot[:],
                        acc2[:],
                        AF.Identity,
                        bias=b_sb[:, HBLK + dd : HBLK + dd + 1],
                    )
                    nc.scalar.dma_start(out=out_d[bi][dd], in_=ot[:])
    return nc


def _cblocks(C):
    """Split [0, C) into near-equal blocks of at most 512 columns."""
    nb = max(1, math.ceil(C / 512))
    base = C // nb
    rem = C - base * nb
    sizes = [base + (1 if i < rem else 0) for i in range(nb)]
    blocks, s = [], 0
    for sz in sizes:
        blocks.append((s, sz))
        s += sz
    return blocks


def kernel(x, Wg, bg, w1, b1, w2, b2, training=0, **_unused):
    from concourse.bass_utils import run_bass_kernel_spmd

    x = np.asarray(x, np.float32)
    Wg = np.asarray(Wg, np.float32)
    bg = np.asarray(bg, np.float32)
    w1 = np.asarray(w1, np.float32)
    b1 = np.asarray(b1, np.float32)
    w2 = np.asarray(w2, np.float32)
    b2 = np.asarray(b2, np.float32)

    gates, top_i, lbl, util = _gate(x, Wg, bg)

    idx = [np.nonzero(gates[:, e])[0] for e in range(E)]
    counts = [len(ix) for ix in idx]
    C = max(128, max(counts))
    cblocks = _cblocks(C)

    xT_bf = x.T.astype(bfloat16)  # [D, B]
    in_maps = []
    for e in range(E):
        xg = np.zeros((D, C), bfloat16)
        xg[:, : counts[e]] = xT_bf[:, idx[e]]
        xg = xg.reshape(DBLK, 128, C)
        w1e = w1[e].astype(bfloat16).reshape(DBLK, 128, H)
        # combined per-partition bias tile: col h = b1[h*128:(h+1)*128],
        # col HBLK+d = b2[d*128:(d+1)*128]
        b_all = np.concatenate(
            [b1[e].reshape(HBLK, 128).T, b2[e].reshape(DBLK, 128).T], axis=1
        ).astype(np.float32)
        w2e = (
            w2[e].astype(bfloat16).reshape(HBLK, 128, D).transpose(1, 0, 2)
        )  # [128, HBLK, D]
        im = {"b": np.ascontiguousarray(b_all)}
        hq = HBLK // 4
        for q in range(4):
            im[f"w2q{q}"] = np.ascontiguousarray(
                w2e[:, q * hq : (q + 1) * hq].reshape(128, hq * D)
            )
        for j, (cs, sz) in enumerate(W1CHUNKS):
            im[f"w1c{j}"] = np.ascontiguousarray(
                w1e[:, :, cs : cs + sz].transpose(1, 0, 2).reshape(128, DBLK * sz)
            )
        for bi, (cstart, csize) in enumerate(cblocks):
            im[f"xT{bi}"] = np.ascontiguousarray(
                xg[:, :, cstart : cstart + csize]
                .transpose(1, 0, 2)
                .reshape(128, DBLK * csize)
            )
        in_maps.append(im)

    nc = _patch_serializer(_build(C, cblocks))
    res = run_bass_kernel_spmd(
        nc,
        in_maps,
        core_ids=list(range(NCORES)),
        trace=TRACE,
        trace_cores=list(range(NCORES)) if TRACE else None,
    )
    LAST_RESULTS["exec_time_ns"] = res.exec_time_ns
    LAST_RESULTS["mean_exec_time_ns"] = res.mean_exec_time_ns
    LAST_RESULTS["res"] = res

    output = np.zeros((B, D), np.float32)
    for e in range(E):
        n = counts[e]
        if n == 0:
            continue
        oT = np.concatenate(
            [
                np.asarray(res.results[e][f"out{bi}"], np.float32).reshape(D, cs)
                for bi, (_, cs) in enumerate(cblocks)
            ],
            axis=1,
        )
        output[idx[e]] += gates[idx[e], e][:, None] * oT[:, :n].T

    return output, gates, top_i, lbl, util


# revision 39
# speedup vs baseline: 1.0066x; 1.0066x over previous
"""MoE (B=2048, D=1024, H=4096, E=8, top-2) — Trainium2 Bass kernel, 8 NeuronCores.

Strategy (expert-parallel, sparse token routing):
  * Host: gating (x @ Wg + bg, top-2, softmax) — 0.01% of the FLOPs. The
    token->expert routing IS the sharding step: core e receives the tokens
    assigned to expert e (gathered + padded to a common capacity C).
  * Device (per core e): outT = (gelu(x_e @ w1[e] + b1[e]) @ w2[e] + b2[e]).T
    computed in bf16 with fp32 PSUM accumulation, fully transposed layout so
    no on-device transposes are needed.
  * Host: weighted scatter-combine of the two expert outputs per token.

The dense reference computes all 8 experts per token; only the top-2 survive
the gates-weighted combine, so routing does 4x less matmul work.
"""

import json
import math

import numpy as np
from ml_dtypes import bfloat16

B, D, H, E, TOP_K = 2048, 1024, 4096, 8, 2
NCORES = 8
DBLK, HBLK = D // 128, H // 128  # 8, 32

# w1 column chunks (small first so the PE can start early; sums to H)
_W1SIZES = [128, 128, 256, 512, 1024, 1024, 1024]
W1CHUNKS = []
_c = 0
for _s in _W1SIZES:
    W1CHUNKS.append((_c, _s))
    _c += _s
W1CIDX = {}  # h-block -> (chunk j, first h-block of chunk, chunk col size)
for _j, (_cs, _sz) in enumerate(W1CHUNKS):
    for _h in range(_cs // 128, (_cs + _sz) // 128):
        W1CIDX[_h] = (_j, _cs // 128, _sz)

TRACE = False  # test.py sets this to capture an NTFF profile / exec_time_ns
LAST_RESULTS = {}  # test.py reads exec_time_ns etc. from here


def _gate(x, Wg, bg):
    """Mirror of the reference gating math in numpy float32."""
    logits = (x @ Wg + bg).astype(np.float32)  # [B, E]
    rows = np.arange(B)
    i1 = np.argmax(logits, axis=1)
    v1 = logits[rows, i1]
    masked = logits.copy()
    masked[rows, i1] = -np.inf
    i2 = np.argmax(masked, axis=1)
    v2 = masked[rows, i2]
    # softmax over the top-2 values (v1 >= v2)
    e2 = np.exp((v2 - v1).astype(np.float32))
    denom = (np.float32(1.0) + e2).astype(np.float32)
    g1 = (np.float32(1.0) / denom).astype(np.float32)
    g2 = (e2 / denom).astype(np.float32)
    gates = np.zeros((B, E), np.float32)
    gates[rows, i1] = g1
    gates[rows, i2] = g2
    top_i = np.stack([i1, i2], axis=1).astype(np.int32)

    # load-balance aux loss
    def _cv(v):
        return np.std(v, ddof=1).astype(np.float32) / (
            np.mean(v, dtype=np.float32) + np.float32(1e-6)
        )

    importance = gates.sum(axis=0, dtype=np.float32)
    m = logits.max(axis=1, keepdims=True)
    ex = np.exp(logits - m)
    sm = ex / ex.sum(axis=1, keepdims=True)
    load = sm.sum(axis=0, dtype=np.float32)
    lbl = np.asarray(_cv(importance) + _cv(load), np.float32)
    util = (gates > 0).astype(np.float32).mean(axis=0, dtype=np.float32)
    return gates, top_i, lbl, util


def _split_multi_waits(mod):
    """Legalize: this walrus accepts at most one sync-wait per instruction.

    Tile's wait assigner can attach several; hoist all but the last onto
    standalone single-wait EventSemaphore instructions on the same engine,
    immediately before the original instruction (same basic block), which
    preserves the per-engine stall semantics exactly.
    """
    for fn in mod["functions"]:
        for blk in fn["blocks"]:
            new_insts = []
            for inst in blk["instructions"]:
                si = inst.get("sync_info") or {}
                waits = si.get("on_wait") or []
                if len(waits) > 1:
                    for k, w in enumerate(waits[:-1]):
                        new_insts.append(
                            {
                                "debug": inst.get("debug", 0),
                                "engine": inst["engine"],
                                "ins": [],
                                "name": f"{inst['name']}-sw{k}",
                                "opcode": "EventSemaphore",
                                "outs": [],
                                "sync_info": {"on_update": [], "on_wait": [w]},
                            }
                        )
                    si["on_wait"] = [waits[-1]]
                new_insts.append(inst)
            blk["instructions"] = new_insts
    return mod


def _patch_serializer(nc):
    orig = nc.to_json_bytes
    nc.to_json_bytes = lambda: json.dumps(_split_multi_waits(json.loads(orig()))).encode()
    return nc


def _build(C, cblocks):
    """Bass/Tile program for one core: out.T = (gelu(x@w1+b1) @ w2 + b2).T.

    Fully transposed dataflow, so no on-device transposes are needed:
      layer 1: psum[h_blk 128, c] += w1[d_blk, h_blk].T @ xT[d_blk, c]  (8 d_blks)
      layer 2: psum[d_blk 128, c] += w2[h_blk, d_blk].T @ hT[h_blk, c]  (32 h_blks)
    Token columns are processed in c-blocks of <=512 (PSUM bank limit).
    All DRAM parameters are host-packed into [128, wide] contiguous blocks.
    """
    import concourse.bass as bass
    import concourse.mybir as mybir
    import concourse.tile as tile
    from concourse.vector_clock import ScopedClock

    bf16, f32 = mybir.dt.bfloat16, mybir.dt.float32
    AF = mybir.ActivationFunctionType

    # Lean kernel epilogue: the stock one is drain -> barrier -> clear all
    # sems -> barrier (~4us). The clears + second barrier only matter when one
    # loaded NEFF is executed repeatedly; here every kernel() call compiles
    # and runs a fresh NEFF exactly once, so drain + one barrier suffices.
    def _lean_drain_and_barrier(self, tick_clock, wait_clock):
        drain_inst = self.nc.sync.drain()
        wait_clock.add_sem_waits(
            drain_inst.ins, ScopedClock({None: tick_clock.global_clock})
        )
        self.nc.all_engine_barrier()
        popped = self.nc._tile_sem_poison_stack.pop()
        assert popped is self._sem_poison

    NB = len(cblocks)

    tile.TileContext._drain_and_barrier = _lean_drain_and_barrier

    nc = bass.Bass()
    # Host pre-blocks everything so each DMA is ONE fully contiguous block
    # landing in one [128, wide] SBUF tile; d-blocks sit side by side in the
    # free dimension and matmuls slice their 128-column windows out. This
    # collapses ~100 DMA issues (~0.6us of sequencer time each) to ~11.
    xT_d = [
        nc.declare_dram_parameter(f"xT{bi}", [128, DBLK * cs], bf16, isOutput=False)
        for bi, (_, cs) in enumerate(cblocks)
    ]
    w1_d = [
        nc.declare_dram_parameter(
            f"w1c{j}", [128, DBLK * csz], bf16, isOutput=False
        )
        for j, (_, csz) in enumerate(W1CHUNKS)
    ]
    w2_d = [
        nc.declare_dram_parameter(f"w2q{q}", [128, (HBLK // 4) * D], bf16, isOutput=False)
        for q in range(4)
    ]
    b_d = nc.declare_dram_parameter("b", [128, HBLK + DBLK], f32, isOutput=False)
    out_d = [
        nc.declare_dram_parameter(f"out{bi}", [DBLK, 128, cs], f32, isOutput=True)
        for bi, (_, cs) in enumerate(cblocks)
    ]

    with tile.TileContext(nc) as tc:
        with (
            tc.tile_pool(name="xp", bufs=1) as xp,
            tc.tile_pool(name="w1p", bufs=1) as w1p,
            tc.tile_pool(name="w2p", bufs=1) as w2p,
            tc.tile_pool(name="bp", bufs=1) as bp,
            tc.tile_pool(name="hp", bufs=1) as hp,
            tc.tile_pool(name="op", bufs=2) as op,
            tc.tile_pool(name="ps", bufs=4, space="PSUM") as ps,
        ):
            # PE warm-up first (highest scheduler priority): ~60 junk matmuls
            # on a memset tile run during the DMA head phase so the HAM clock
            # gate is at 8/8 when real work lands.
            junk = bp.tile([128, 64], bf16, tag="junk")
            nc.vector.memset(junk[:], 0.0)
            jps = ps.tile([64, 64], f32, tag="ps1")
            for _ in range(60):
                nc.tensor.matmul(jps[:], junk[:], junk[:], start=True, stop=True)

            # ALL input loads go on the single SP HWDGE ring in exact PE
            # consumption order — the ring is FIFO, so this IS the priority
            # schedule: xT block 0, biases, w1 chunks, xT rest, then w2 in
            # four quarters (each unblocks 8 h-blocks of layer 2). Outputs go
            # on the ACT ring.
            x_sb = [None] * NB
            t = xp.tile([128, DBLK * cblocks[0][1]], bf16, tag="x0", name="x0")
            nc.sync.dma_start(out=t[:], in_=xT_d[0][:])
            x_sb[0] = t
            w1_sb = []
            b_sb = None
            for j, (_, csz) in enumerate(W1CHUNKS):
                t = w1p.tile([128, DBLK * csz], bf16, tag=f"w1_{j}", name=f"w1_{j}")
                nc.sync.dma_start(out=t[:], in_=w1_d[j][:])
                w1_sb.append(t)
                if j == 0:
                    b_sb = bp.tile([128, HBLK + DBLK], f32, tag="b")
                    nc.sync.dma_start(out=b_sb[:], in_=b_d[:])
            for bi in range(1, NB):
                t = xp.tile([128, DBLK * cblocks[bi][1]], bf16, tag=f"x{bi}", name=f"x{bi}")
                nc.sync.dma_start(out=t[:], in_=xT_d[bi][:])
                x_sb[bi] = t
            w2_sb = []
            for q in range(4):
                t = w2p.tile([128, (HBLK // 4) * D], bf16, tag=f"w2q{q}", name=f"w2q{q}")
                nc.sync.dma_start(out=t[:], in_=w2_d[q][:])
                w2_sb.append(t)

            def w1_lhsT(d, h):
                j, h0, csz = W1CIDX[h]
                base = d * csz + (h - h0) * 128
                return w1_sb[j][:, base : base + 128]

            for bi, (cstart, csize) in enumerate(cblocks):
                # layer 1: hT[h,c] = gelu(sum_d w1[d,h].T @ xT[d,c] + b1[h])
                h_tiles = []
                for h in range(HBLK):
                    acc = ps.tile([128, csize], f32, tag="ps1")
                    for d in range(DBLK):
                        nc.tensor.matmul(
                            acc[:],
                            w1_lhsT(d, h),
                            x_sb[bi][:, d * csize : (d + 1) * csize],
                            start=(d == 0),
                            stop=(d == DBLK - 1),
                        )
                    ht = hp.tile([128, csize], bf16, tag=f"h{h}")
                    nc.scalar.activation(ht[:], acc[:], AF.Gelu, bias=b_sb[:, h : h + 1])
                    h_tiles.append(ht)
                # layer 2: outT[d,c] = sum_h w2[h,d].T @ hT[h,c] + b2[d]
                for dd in range(DBLK):
                    acc2 = ps.tile([128, csize], f32, tag="ps2")
                    for h in range(HBLK):
                        hq, hr = divmod(h, HBLK // 4)
                        nc.tensor.matmul(
                            acc2[:],
                            w2_sb[hq][:, hr * D + dd * 128 : hr * D + dd * 128 + 128],
                            h_tiles[h][:],
                            start=(h == 0),
                            stop=(h == HBLK - 1),
                        )
                    ot = op.tile([128, csize], f32, tag="ot")
                    nc.scalar.activation(
                        ot[:],
                        acc2[:],
                        AF.Identity,
                        bias=b_sb[:, HBLK + dd : HBLK + dd + 1],
                    )
                    nc.scalar.dma_start(out=out_d[bi][dd], in_=ot[:])
    return nc


def _cblocks(C):
    """Split [0, C) into near-equal blocks of at most 512 columns."""
    nb = max(1, math.ceil(C / 512))
    base = C // nb
    rem = C - base * nb
    sizes = [base + (1 if i < rem else 0) for i in range(nb)]
    blocks, s = [], 0
    for sz in sizes:
        blocks.append((s, sz))
        s += sz
    return blocks


def kernel(x, Wg, bg, w1, b1, w2, b2, training=0, **_unused):
    from concourse.bass_utils import run_bass_kernel_spmd

    x = np.asarray(x, np.float32)
    Wg = np.asarray(Wg, np.float32)
    bg = np.asarray(bg, np.float32)
    w1 = np.asarray(w1, np.float32)
    b1 = np.asarray(b1, np.float32)
    w2 = np.asarray(w2, np.float32)
    b2 = np.asarray(b2, np.float32)

    gates, top_i, lbl, util = _gate(x, Wg, bg)

    idx = [np.nonzero(gates[:, e])[0] for e in range(E)]
    counts = [len(ix) for ix in idx]
    C = max(128, max(counts))
    cblocks = _cblocks(C)

    xT_bf = x.T.astype(bfloat16)  # [D, B]
    in_maps = []
    for e in range(E):
        xg = np.zeros((D, C), bfloat16)
        xg[:, : counts[e]] = xT_bf[:, idx[e]]
        xg = xg.reshape(DBLK, 128, C)
        w1e = w1[e].astype(bfloat16).reshape(DBLK, 128, H)
        # combined per-partition bias tile: col h = b1[h*128:(h+1)*128],
        # col HBLK+d = b2[d*128:(d+1)*128]
        b_all = np.concatenate(
            [b1[e].reshape(HBLK, 128).T, b2[e].reshape(DBLK, 128).T], axis=1
        ).astype(np.float32)
        w2e = (
            w2[e].astype(bfloat16).reshape(HBLK, 128, D).transpose(1, 0, 2)
        )  # [128, HBLK, D]
        im = {"b": np.ascontiguousarray(b_all)}
        hq = HBLK // 4
        for q in range(4):
            im[f"w2q{q}"] = np.ascontiguousarray(
                w2e[:, q * hq : (q + 1) * hq].reshape(128, hq * D)
            )
        for j, (cs, sz) in enumerate(W1CHUNKS):
            im[f"w1c{j}"] = np.ascontiguousarray(
                w1e[:, :, cs : cs + sz].transpose(1, 0, 2).reshape(128, DBLK * sz)
            )
        for bi, (cstart, csize) in enumerate(cblocks):
            im[f"xT{bi}"] = np.ascontiguousarray(
                xg[:, :, cstart : cstart + csize]
                .transpose(1, 0, 2)
                .reshape(128, DBLK * csize)
            )
        in_maps.append(im)

    nc = _patch_serializer(_build(C, cblocks))
    res = run_bass_kernel_spmd(
        nc,
        in_maps,
        core_ids=list(range(NCORES)),
        trace=TRACE,
        trace_cores=list(range(NCORES)) if TRACE else None,
    )
    LAST_RESULTS["exec_time_ns"] = res.exec_time_ns
    LAST_RESULTS["mean_exec_time_ns"] = res.mean_exec_time_ns
    LAST_RESULTS["res"] = res

    output = np.zeros((B, D), np.float32)
    for e in range(E):
        n = counts[e]
        if n == 0:
            continue
        oT = np.concatenate(
            [
                np.asarray(res.results[e][f"out{bi}"], np.float32).reshape(D, cs)
                for bi, (_, cs) in enumerate(cblocks)
            ],
            axis=1,
        )
        output[idx[e]] += gates[idx[e], e][:, None] * oT[:, :n].T

    return output, gates, top_i, lbl, util


# revision 42
# speedup vs baseline: 1.0124x; 1.0058x over previous
"""MoE (B=2048, D=1024, H=4096, E=8, top-2) — Trainium2 Bass kernel, 8 NeuronCores.

Strategy (expert-parallel, sparse token routing):
  * Host: gating (x @ Wg + bg, top-2, softmax) — 0.01% of the FLOPs. The
    token->expert routing IS the sharding step: core e receives the tokens
    assigned to expert e (gathered + padded to a common capacity C).
  * Device (per core e): outT = (gelu(x_e @ w1[e] + b1[e]) @ w2[e] + b2[e]).T
    computed in bf16 with fp32 PSUM accumulation, fully transposed layout so
    no on-device transposes are needed.
  * Host: weighted scatter-combine of the two expert outputs per token.

The dense reference computes all 8 experts per token; only the top-2 survive
the gates-weighted combine, so routing does 4x less matmul work.
"""

import json
import math

import numpy as np
from ml_dtypes import bfloat16

B, D, H, E, TOP_K = 2048, 1024, 4096, 8, 2
NCORES = 8
DBLK, HBLK = D // 128, H // 128  # 8, 32

# w1 column chunks (small first so the PE can start early; sums to H)
_W1SIZES = [128, 128, 256, 512, 1024, 1024, 1024]
W1CHUNKS = []
_c = 0
for _s in _W1SIZES:
    W1CHUNKS.append((_c, _s))
    _c += _s
W1CIDX = {}  # h-block -> (chunk j, first h-block of chunk, chunk col size)
for _j, (_cs, _sz) in enumerate(W1CHUNKS):
    for _h in range(_cs // 128, (_cs + _sz) // 128):
        W1CIDX[_h] = (_j, _cs // 128, _sz)

TRACE = False  # test.py sets this to capture an NTFF profile / exec_time_ns
LAST_RESULTS = {}  # test.py reads exec_time_ns etc. from here


def _gate(x, Wg, bg):
    """Mirror of the reference gating math in numpy float32."""
    logits = (x @ Wg + bg).astype(np.float32)  # [B, E]
    rows = np.arange(B)
    i1 = np.argmax(logits, axis=1)
    v1 = logits[rows, i1]
    masked = logits.copy()
    masked[rows, i1] = -np.inf
    i2 = np.argmax(masked, axis=1)
    v2 = masked[rows, i2]
    # softmax over the top-2 values (v1 >= v2)
    e2 = np.exp((v2 - v1).astype(np.float32))
    denom = (np.float32(1.0) + e2).astype(np.float32)
    g1 = (np.float32(1.0) / denom).astype(np.float32)
    g2 = (e2 / denom).astype(np.float32)
    gates = np.zeros((B, E), np.float32)
    gates[rows, i1] = g1
    gates[rows, i2] = g2
    top_i = np.stack([i1, i2], axis=1).astype(np.int32)

    # load-balance aux loss
    def _cv(v):
        return np.std(v, ddof=1).astype(np.float32) / (
            np.mean(v, dtype=np.float32) + np.float32(1e-6)
        )

    importance = gates.sum(axis=0, dtype=np.float32)
    m = logits.max(axis=1, keepdims=True)
    ex = np.exp(logits - m)
    sm = ex / ex.sum(axis=1, keepdims=True)
    load = sm.sum(axis=0, dtype=np.float32)
    lbl = np.asarray(_cv(importance) + _cv(load), np.float32)
    util = (gates > 0).astype(np.float32).mean(axis=0, dtype=np.float32)
    return gates, top_i, lbl, util


def _split_multi_waits(mod):
    """Legalize: this walrus accepts at most one sync-wait per instruction.

    Tile's wait assigner can attach several; hoist all but the last onto
    standalone single-wait EventSemaphore instructions on the same engine,
    immediately before the original instruction (same basic block), which
    preserves the per-engine stall semantics exactly.
    """
    for fn in mod["functions"]:
        for blk in fn["blocks"]:
            new_insts = []
            for inst in blk["instructions"]:
                si = inst.get("sync_info") or {}
                waits = si.get("on_wait") or []
                if len(waits) > 1:
                    for k, w in enumerate(waits[:-1]):
                        new_insts.append(
                            {
                                "debug": inst.get("debug", 0),
                                "engine": inst["engine"],
                                "ins": [],
                                "name": f"{inst['name']}-sw{k}",
                                "opcode": "EventSemaphore",
                                "outs": [],
                                "sync_info": {"on_update": [], "on_wait": [w]},
                            }
                        )
                    si["on_wait"] = [waits[-1]]
                new_insts.append(inst)
            blk["instructions"] = new_insts
    return mod


def _patch_serializer(nc):
    orig = nc.to_json_bytes
    nc.to_json_bytes = lambda: json.dumps(_split_multi_waits(json.loads(orig()))).encode()
    return nc


def _build(C, cblocks):
    """Bass/Tile program for one core: out.T = (gelu(x@w1+b1) @ w2 + b2).T.

    Fully transposed dataflow, so no on-device transposes are needed:
      layer 1: psum[h_blk 128, c] += w1[d_blk, h_blk].T @ xT[d_blk, c]  (8 d_blks)
      layer 2: psum[d_blk 128, c] += w2[h_blk, d_blk].T @ hT[h_blk, c]  (32 h_blks)
    Token columns are processed in c-blocks of <=512 (PSUM bank limit).
    All DRAM parameters are host-packed into [128, wide] contiguous blocks.
    """
    import concourse.bass as bass
    import concourse.mybir as mybir
    import concourse.tile as tile

    bf16, f32 = mybir.dt.bfloat16, mybir.dt.float32
    AF = mybir.ActivationFunctionType


    NB = len(cblocks)

    nc = bass.Bass()
    # Host pre-blocks everything so each DMA is ONE fully contiguous block
    # landing in one [128, wide] SBUF tile; d-blocks sit side by side in the
    # free dimension and matmuls slice their 128-column windows out. This
    # collapses ~100 DMA issues (~0.6us of sequencer time each) to ~11.
    xT_d = [
        nc.declare_dram_parameter(f"xT{bi}", [128, DBLK * cs], bf16, isOutput=False)
        for bi, (_, cs) in enumerate(cblocks)
    ]
    w1_d = [
        nc.declare_dram_parameter(
            f"w1c{j}", [128, DBLK * csz], bf16, isOutput=False
        )
        for j, (_, csz) in enumerate(W1CHUNKS)
    ]
    w2_d = [
        nc.declare_dram_parameter(f"w2q{q}", [128, (HBLK // 4) * D], bf16, isOutput=False)
        for q in range(4)
    ]
    b_d = nc.declare_dram_parameter("b", [128, HBLK + DBLK], f32, isOutput=False)
    out_d = [
        nc.declare_dram_parameter(f"out{bi}", [DBLK, 128, cs], f32, isOutput=True)
        for bi, (_, cs) in enumerate(cblocks)
    ]

    with tile.TileContext(nc) as tc:
        with (
            tc.tile_pool(name="xp", bufs=1) as xp,
            tc.tile_pool(name="w1p", bufs=1) as w1p,
            tc.tile_pool(name="w2p", bufs=1) as w2p,
            tc.tile_pool(name="bp", bufs=1) as bp,
            tc.tile_pool(name="hp", bufs=1) as hp,
            tc.tile_pool(name="op", bufs=2) as op,
            tc.tile_pool(name="ps", bufs=4, space="PSUM") as ps,
        ):
            # PE warm-up first (highest scheduler priority): ~60 junk matmuls
            # on a memset tile run during the DMA head phase so the HAM clock
            # gate is at 8/8 when real work lands.
            junk = bp.tile([128, 64], bf16, tag="junk")
            nc.vector.memset(junk[:], 0.0)
            jps = ps.tile([64, 64], f32, tag="ps1")
            for _ in range(60):
                nc.tensor.matmul(jps[:], junk[:], junk[:], start=True, stop=True)

            # ALL input loads go on the single SP HWDGE ring in exact PE
            # consumption order — the ring is FIFO, so this IS the priority
            # schedule: xT block 0, biases, w1 chunks, xT rest, then w2 in
            # four quarters (each unblocks 8 h-blocks of layer 2). Outputs go
            # on the ACT ring.
            x_sb = [None] * NB
            t = xp.tile([128, DBLK * cblocks[0][1]], bf16, tag="x0", name="x0")
            nc.sync.dma_start(out=t[:], in_=xT_d[0][:])
            x_sb[0] = t
            w1_sb = []
            b_sb = None
            for j, (_, csz) in enumerate(W1CHUNKS):
                t = w1p.tile([128, DBLK * csz], bf16, tag=f"w1_{j}", name=f"w1_{j}")
                nc.sync.dma_start(out=t[:], in_=w1_d[j][:])
                w1_sb.append(t)
                if j == 0:
                    b_sb = bp.tile([128, HBLK + DBLK], f32, tag="b")
                    nc.sync.dma_start(out=b_sb[:], in_=b_d[:])
            for bi in range(1, NB):
                t = xp.tile([128, DBLK * cblocks[bi][1]], bf16, tag=f"x{bi}", name=f"x{bi}")
                nc.sync.dma_start(out=t[:], in_=xT_d[bi][:])
                x_sb[bi] = t
            w2_sb = []
            for q in range(4):
                t = w2p.tile([128, (HBLK // 4) * D], bf16, tag=f"w2q{q}", name=f"w2q{q}")
                nc.sync.dma_start(out=t[:], in_=w2_d[q][:])
                w2_sb.append(t)

            def w1_lhsT(d, h):
                j, h0, csz = W1CIDX[h]
                base = d * csz + (h - h0) * 128
                return w1_sb[j][:, base : base + 128]

            for bi, (cstart, csize) in enumerate(cblocks):
                # layer 1: hT[h,c] = gelu(sum_d w1[d,h].T @ xT[d,c] + b1[h])
                h_tiles = []
                for h in range(HBLK):
                    acc = ps.tile([128, csize], f32, tag="ps1")
                    for d in range(DBLK):
                        nc.tensor.matmul(
                            acc[:],
                            w1_lhsT(d, h),
                            x_sb[bi][:, d * csize : (d + 1) * csize],
                            start=(d == 0),
                            stop=(d == DBLK - 1),
                        )
                    ht = hp.tile([128, csize], bf16, tag=f"h{h}")
                    nc.scalar.activation(ht[:], acc[:], AF.Gelu, bias=b_sb[:, h : h + 1])
                    h_tiles.append(ht)
                # layer 2: outT[d,c] = sum_h w2[h,d].T @ hT[h,c] + b2[d]
                for dd in range(DBLK):
                    acc2 = ps.tile([128, csize], f32, tag="ps2")
                    for h in range(HBLK):
                        hq, hr = divmod(h, HBLK // 4)
                        nc.tensor.matmul(
                            acc2[:],
                            w2_sb[hq][:, hr * D + dd * 128 : hr * D + dd * 128 + 128],
                            h_tiles[h][:],
                            start=(h == 0),
                            stop=(h == HBLK - 1),
                        )
                    ot = op.tile([128, csize], f32, tag="ot")
                    nc.scalar.activation(
                        ot[:],
                        acc2[:],
                        AF.Identity,
                        bias=b_sb[:, HBLK + dd : HBLK + dd + 1],
                    )
                    nc.scalar.dma_start(out=out_d[bi][dd], in_=ot[:])
    return nc


def _cblocks(C):
    """Split [0, C) into near-equal blocks of at most 512 columns."""
    nb = max(1, math.ceil(C / 512))
    base = C // nb
    rem = C - base * nb
    sizes = [base + (1 if i < rem else 0) for i in range(nb)]
    blocks, s = [], 0
    for sz in sizes:
        blocks.append((s, sz))
        s += sz
    return blocks


def kernel(x, Wg, bg, w1, b1, w2, b2, training=0, **_unused):
    from concourse.bass_utils import run_bass_kernel_spmd

    x = np.asarray(x, np.float32)
    Wg = np.asarray(Wg, np.float32)
    bg = np.asarray(bg, np.float32)
    w1 = np.asarray(w1, np.float32)
    b1 = np.asarray(b1, np.float32)
    w2 = np.asarray(w2, np.float32)
    b2 = np.asarray(b2, np.float32)

    gates, top_i, lbl, util = _gate(x, Wg, bg)

    idx = [np.nonzero(gates[:, e])[0] for e in range(E)]
    counts = [len(ix) for ix in idx]
    C = max(128, max(counts))
    cblocks = _cblocks(C)

    xT_bf = x.T.astype(bfloat16)  # [D, B]
    in_maps = []
    for e in range(E):
        xg = np.zeros((D, C), bfloat16)
        xg[:, : counts[e]] = xT_bf[:, idx[e]]
        xg = xg.reshape(DBLK, 128, C)
        w1e = w1[e].astype(bfloat16).reshape(DBLK, 128, H)
        # combined per-partition bias tile: col h = b1[h*128:(h+1)*128],
        # col HBLK+d = b2[d*128:(d+1)*128]
        b_all = np.concatenate(
            [b1[e].reshape(HBLK, 128).T, b2[e].reshape(DBLK, 128).T], axis=1
        ).astype(np.float32)
        w2e = (
            w2[e].astype(bfloat16).reshape(HBLK, 128, D).transpose(1, 0, 2)
        )  # [128, HBLK, D]
        im = {"b": np.ascontiguousarray(b_all)}
        hq = HBLK // 4
        for q in range(4):
            im[f"w2q{q}"] = np.ascontiguousarray(
                w2e[:, q * hq : (q + 1) * hq].reshape(128, hq * D)
            )
        for j, (cs, sz) in enumerate(W1CHUNKS):
            im[f"w1c{j}"] = np.ascontiguousarray(
                w1e[:, :, cs : cs + sz].transpose(1, 0, 2).reshape(128, DBLK * sz)
            )
        for bi, (cstart, csize) in enumerate(cblocks):
            im[f"xT{bi}"] = np.ascontiguousarray(
                xg[:, :, cstart : cstart + csize]
                .transpose(1, 0, 2)
                .reshape(128, DBLK * csize)
            )
        in_maps.append(im)

    nc = _patch_serializer(_build(C, cblocks))
    res = run_bass_kernel_spmd(
        nc,
        in_maps,
        core_ids=list(range(NCORES)),
        trace=TRACE,
        trace_cores=list(range(NCORES)) if TRACE else None,
    )
    LAST_RESULTS["exec_time_ns"] = res.exec_time_ns
    LAST_RESULTS["mean_exec_time_ns"] = res.mean_exec_time_ns
    LAST_RESULTS["res"] = res

    output = np.zeros((B, D), np.float32)
    for e in range(E):
        n = counts[e]
        if n == 0:
            continue
        oT = np.concatenate(
            [
                np.asarray(res.results[e][f"out{bi}"], np.float32).reshape(D, cs)
                for bi, (_, cs) in enumerate(cblocks)
            ],
            axis=1,
        )
        output[idx[e]] += gates[idx[e], e][:, None] * oT[:, :n].T

    return output, gates, top_i, lbl, util
